# revision 2
# baseline (speedup 1.0000x reference)
import numpy as np
import ml_dtypes

N = 50000
F = 64
E = 128
Q = 8
S = 2048
NC = 8
NPC = N // NC          # 6250 clauses per core
NPAD = 6400            # 25 * 256
NSC = 25               # super-chunks of 256 clauses (DoubleRow)
NCHUNK = 50            # 128-chunks
SB = 4                 # psum banks of 512 steps each
ST = 32                # stationary cols: Ehi(8) Elo(8) Ghi(8) Glo(8)
GS = 0.125             # scale on x for G so fp8 never saturates (|G|<448)
ENTROPY_COEF = 0.1
NG = 5                 # super-chunk groups for PE interleaving
GSC = NSC // NG        # super-chunks per group

_PROG = None


def _build_prog():
    import sys
    if "/opt/trn_rl_repo" not in sys.path:
        sys.path.insert(0, "/opt/trn_rl_repo")
    from concourse import bass, bacc, tile, mybir

    f32 = mybir.dt.float32
    bf16 = mybir.dt.bfloat16
    f8 = mybir.dt.float8e4
    AF = mybir.ActivationFunctionType
    ALU = mybir.AluOpType
    DR = mybir.MatmulPerfMode.DoubleRow

    nc = bacc.Bacc("TRN2")
    fw_d = nc.dram_tensor("fw", [F, E + NPAD], bf16, kind="ExternalInput")
    wb_d = nc.dram_tensor("wb", [E, Q + 1], f32, kind="ExternalInput")
    maskT_d = nc.dram_tensor("maskT", [128, NSC, 2, S], f8, kind="ExternalInput")
    stats_d = nc.dram_tensor("stats", [ST, S], f32, kind="ExternalOutput")
    xall_d = nc.dram_tensor("xall", [E, NCHUNK * Q], f32, kind="ExternalOutput")

    with tile.TileContext(nc) as tc:
        with (
            tc.tile_pool(name="const", bufs=1) as constp,
            tc.tile_pool(name="big", bufs=1) as bigp,
            tc.tile_pool(name="mask", bufs=1) as maskp,
            tc.tile_pool(name="ps", bufs=1, space=bass.MemorySpace.PSUM) as ps,
        ):
            wb_sb = constp.tile([E, Q + 1], f32)
            k2t_sb = constp.tile([E, Q], bf16)
            scr_sb = constp.tile([1, 1], f32)
            warm_sb = constp.tile([1, 256], bf16)

            fw_sb = bigp.tile([F, E + NPAD], bf16)
            ht_sb = bigp.tile([E, NPAD], bf16)
            xall_sb = bigp.tile([E, NCHUNK * Q], f32)
            e_sb = bigp.tile([E, NSC, 2, Q], f32)
            xs_sb = bigp.tile([E, NSC, 2, Q], f32)
            g_sb = bigp.tile([E, NSC, 2, Q], f32)
            ehi_sb = bigp.tile([E, NSC, 2, Q], f32)
            ghi_sb = bigp.tile([E, NSC, 2, Q], f32)
            stat_sb = bigp.tile([E, NSC, 2, ST], f8)
            stats_sb = bigp.tile([ST, SB * 512], f32)

            w1_sb = fw_sb[:, 0:E]
            fvt_sb = fw_sb[:, E:E + NPAD]
            b1_sb = wb_sb[:, Q:Q + 1]

            # Mask stream first on the SP ring: big blocks early (they arrive
            # while PE still does the embedder), single-super-chunk blocks at
            # the tail so the last stats matmuls wait on only 512KB.
            MBLK = [(0, 5), (5, 5), (10, 5), (15, 4),
                    (19, 1), (20, 1), (21, 1), (22, 1), (23, 1), (24, 1)]
            mts = {}
            for s0, w in MBLK:
                mt = maskp.tile([128, w, 2, S], f8, tag=f"m{s0}")
                nc.sync.dma_start(mt[:], maskT_d[:, s0:s0 + w, :, :])
                mts[s0] = (w, mt)

            def mslice(sc, b):
                for s0, (w, mt) in mts.items():
                    if s0 <= sc < s0 + w:
                        return mt[:, sc - s0, :, 512 * b:512 * (b + 1)]

            # fw/wb ride the ACT ring so the whole SP ring belongs to the mask
            nc.scalar.dma_start(fw_sb[:], fw_d[:])
            nc.scalar.dma_start(wb_sb[:], wb_d[:])

            # ACT absorber for the wb DMA semaphore, then k2t copy on ACT so
            # the px matmuls depend on a single engine (ACT) only.
            nc.scalar.activation(scr_sb[:], wb_sb[0:1, 0:1], AF.Relu)
            nc.scalar.activation(k2t_sb[:], wb_sb[:, 0:Q], AF.Copy)

            # PE p-state warm-up: the tensor engine ramps 1.2->2.4 GHz only
            # after ~3us of continuous execution. Run dummy matmuls on a
            # zeroed scratch while waiting for fw so the real work starts hot.
            nc.gpsimd.memset(warm_sb[:], 0.0)
            xps = ps.tile([E, 512], f32, tag="x", bufs=1, name="x")
            for i in range(12):
                nc.tensor.matmul(xps[0:1, 0:256], warm_sb[0:1, 0:1],
                                 warm_sb[0:1, 0:256], start=True, stop=True)

            # hT = relu(W1.T @ fvT + b1)   [E, NPAD] bf16
            for j in range((NPAD + 511) // 512):
                c0 = 512 * j
                cw = min(512, NPAD - c0)
                ph = ps.tile([E, 512], f32, tag="w", bufs=2, name="w")
                nc.tensor.matmul(ph[:, :cw], w1_sb, fvt_sb[:, c0:c0 + cw],
                                 start=True, stop=True)
                nc.scalar.activation(ht_sb[:, c0:c0 + cw], ph[:, :cw], AF.Relu,
                                     bias=b1_sb, scale=1.0)

            stats_ps = [ps.tile([ST, 512], f32, tag=f"s{b}", bufs=1, name=f"s{b}")
                        for b in range(SB)]

            def emit_x(g):
                # x'[n,q] = hT_chunk.T @ K2T (c_q dropped: softmax shift-inv.)
                for c in range(2 * GSC * g, 2 * GSC * (g + 1)):
                    nc.tensor.matmul(xps[:, Q * c:Q * (c + 1)],
                                     ht_sb[:, 128 * c:128 * (c + 1)], k2t_sb[:],
                                     start=True, stop=True)

            def emit_prep(g):
                sl = slice(2 * GSC * Q * g, 2 * GSC * Q * (g + 1))
                sc = slice(GSC * g, GSC * (g + 1))
                nc.scalar.activation(e_sb[:, sc, :, :], xps[:, sl], AF.Exp)
                nc.scalar.activation(xs_sb[:, sc, :, :], xps[:, sl], AF.Copy,
                                     scale=GS)
                nc.vector.tensor_tensor(g_sb[:, sc], xs_sb[:, sc], e_sb[:, sc],
                                        ALU.mult)
                # hi/lo fp8 split: value = hi + lo, ~2^-8 combined rel err
                nc.vector.tensor_copy(stat_sb[:, sc, :, 0:Q], e_sb[:, sc])
                nc.vector.tensor_copy(ehi_sb[:, sc], stat_sb[:, sc, :, 0:Q])
                nc.vector.tensor_tensor(stat_sb[:, sc, :, Q:2 * Q], e_sb[:, sc],
                                        ehi_sb[:, sc], ALU.subtract)
                nc.vector.tensor_copy(stat_sb[:, sc, :, 2 * Q:3 * Q], g_sb[:, sc])
                nc.vector.tensor_copy(ghi_sb[:, sc], stat_sb[:, sc, :, 2 * Q:3 * Q])
                nc.vector.tensor_tensor(stat_sb[:, sc, :, 3 * Q:4 * Q], g_sb[:, sc],
                                        ghi_sb[:, sc], ALU.subtract)

            def emit_stats(g, final=False):
                # stats[32,S] += stat_chunk.T @ maskT_chunk, fp8 DoubleRow K=256
                for k in range(GSC * g, GSC * (g + 1)):
                    last = final and k == NSC - 1
                    for b in range(SB):
                        nc.tensor.matmul(stats_ps[b][:, :], stat_sb[:, k, :, :],
                                         mslice(k, b), start=(k == 0), stop=last,
                                         perf_mode=DR, skip_group_check=True)
                        if last:
                            # per-bank evacuation split over ACT and DVE;
                            # separate psum tiles keep banks independent
                            dst = stats_sb[:, 512 * b:512 * (b + 1)]
                            if b % 2 == 0:
                                nc.scalar.activation(dst, stats_ps[b][:, :],
                                                     AF.Copy)
                            else:
                                nc.vector.tensor_copy(dst, stats_ps[b][:, :])

            # 2-group lookahead at the start so stats never stall on prep
            emit_x(0); emit_prep(0)
            emit_x(1); emit_prep(1)
            emit_x(2); emit_prep(2)
            emit_stats(0)
            emit_x(3); emit_prep(3)
            emit_stats(1)
            emit_x(4); emit_prep(4)
            # xall is final now; ship it (SP ring idle) while stats run
            nc.scalar.activation(xall_sb[:], xps[:, 0:NCHUNK * Q], AF.Copy)
            nc.sync.dma_start(xall_d[:], xall_sb[:])
            emit_stats(2)
            emit_stats(3)
            emit_stats(4, final=True)
            nc.sync.dma_start(stats_d[:, 0:1024], stats_sb[:, 0:1024])
            nc.sync.dma_start(stats_d[:, 1024:2048], stats_sb[:, 1024:2048])

    nc.finalize()
    return nc


def _get_prog():
    global _PROG
    if _PROG is None:
        _PROG = _build_prog()
    return _PROG


def _prep(feature_vecs, W1, b1, W2, b2, keys, mask):
    m8 = mask.view(np.uint8) if mask.dtype == np.bool_ else mask.astype(np.uint8)
    m8 = m8 * np.uint8(0x38)               # fp8e4m3 bit pattern of 1.0
    mT = np.ascontiguousarray(m8.T)        # [N, S]

    wb = np.zeros((E, Q + 1), np.float32)
    wb[:, 0:Q] = (np.asarray(W2, np.float64) @ np.asarray(keys, np.float64).T
                  ).astype(np.float32)     # K2T[e,q]
    wb[:, Q] = np.asarray(b1, np.float32)

    w1b = np.asarray(W1).astype(ml_dtypes.bfloat16)

    in_maps = []
    for d in range(NC):
        sl = slice(d * NPC, (d + 1) * NPC)
        fw = np.zeros((F, E + NPAD), ml_dtypes.bfloat16)
        fw[:, 0:E] = w1b
        fw[:, E:E + NPC] = feature_vecs[sl].T.astype(ml_dtypes.bfloat16)
        mt = np.zeros((NPAD, S), np.uint8)
        mt[:NPC] = mT[sl]
        mt4 = np.ascontiguousarray(
            mt.reshape(NSC, 2, 128, S).transpose(2, 0, 1, 3))
        in_maps.append({
            "fw": fw,
            "wb": wb,
            "maskT": mt4.view(ml_dtypes.float8_e4m3),
        })
    return in_maps


def kernel(feature_vecs, W1, b1, W2, b2, keys, rewards, mask, queue_idx, sel_idx):
    import sys
    if "/opt/trn_rl_repo" not in sys.path:
        sys.path.insert(0, "/opt/trn_rl_repo")
    from concourse.bass_utils import run_bass_kernel_spmd

    nc = _get_prog()
    in_maps = _prep(feature_vecs, W1, b1, W2, b2, keys, mask)
    res = run_bass_kernel_spmd(nc, in_maps, list(range(NC))).results

    qs = np.asarray(queue_idx).astype(np.int64)
    ar = np.arange(S)
    Z = np.zeros(S, np.float64)
    S1 = np.zeros(S, np.float64)
    cnt = np.asarray(mask).sum(axis=1, dtype=np.float64)
    for d in range(NC):
        st = res[d]["stats"].astype(np.float64)
        Z += st[qs, ar] + st[Q + qs, ar]
        S1 += st[2 * Q + qs, ar] + st[3 * Q + qs, ar]
    S1 /= GS

    xall = np.stack([res[d]["xall"] for d in range(NC)]).astype(np.float64)
    sel = np.asarray(sel_idx).astype(np.int64)
    d_arr = sel // NPC
    nloc = sel % NPC
    x_sel = xall[d_arr, nloc % 128, (nloc // 128) * Q + qs]

    logZ = np.log(Z)
    ce = logZ - x_sel
    me = (S1 / Z - logZ) / np.log(cnt)
    loss = (np.asarray(rewards, np.float64) * ce).sum() + ENTROPY_COEF * me.sum()
    return np.array([loss], dtype=np.float32)
